# revision 6
# baseline (speedup 1.0000x reference)
"""Contrastive loss (GRACE-style semi_loss pair) on 8 trn2 NeuronCores.

Math (reference):
    a = z1 / ||z1||_row ; b = z2 / ||z2||_row         (N=8192, D=512)
    refl    = exp(a @ a.T / tau) ; between = exp(a @ b.T / tau)
    l1_i = -log(between_ii / (refl.sum(1) + between.sum(1) - refl_ii))
    l2   = same with (z2, z1) swapped
    loss = mean(0.5 * (l1 + l2))

Key identities used:
  - between2 (for l2) = between.T, so its row sums are COLUMN sums of
    exp(a@b.T/tau) -> one cross-core ReduceScatter of [8192] floats,
    no 4th matmul.
  - refl_ii = exp(1/tau) exactly (rows are unit-norm).
  - between_ii needs only dab_i = a_i . b_i (computed row-wise in fp32).
  - l1_i = log(denom1_i) - dab_i/tau ; l2_i = log(denom2_i) - dab_i/tau.

Sharding: data-parallel rows. Every core receives the full z (row-major,
for norms) and full zT (d-major, the matmul moving operand) plus its own
1024-row slice (stationary side). Per core:
  phase A: stream z row-major once, fused square+row-sum on DVE ->
           1/norm = exp(-0.5*ln(sumsq)) on ACT (one table set, loaded
           once; no sqrt/rsqrt sets) -> spill 1/norm to DRAM flat.
  phase B: per 512-column chunk: broadcast 1/norm across partitions with
           a K=1 ones-matmul into PSUM, scale the streamed zT tiles on
           DVE (fp32 -> bf16) into the persistent operand tiles.
  main:    per (chunk n, local 128-row block m): 3 accumulation groups of
           4 bf16 matmuls (S_aa, S_ab, S_bb), fused exp+row-sum on ACT
           (aa/bb exp'd in place in PSUM), exp(S_ab) kept in bf16 for
           column sums accumulated on DVE, folded once per chunk by a
           ones-matmul.
  tail:    ReduceScatter(column sums), denominators, Ln, per-core partial
           -> AllReduce scalar -> loss.
"""

import numpy as np
from contextlib import ExitStack

import concourse.bass as bass
import concourse.tile as tile
from concourse import bacc, mybir
from concourse.bass_utils import run_bass_kernel_spmd

N = 8192
D = 512
P = 128
NCORES = 8
LOCAL = N // NCORES            # 1024 rows per core
M_CH = LOCAL // P              # 8 local row chunks of 128
N_CH = N // 512                # 16 column chunks of 512
KC = D // P                    # 4 contraction chunks of 128
TAU = 0.4
EXPD = float(np.exp(1.0 / TAU))   # diagonal of exp(S_aa/tau): rows unit-norm

FP32 = mybir.dt.float32
BF16 = mybir.dt.bfloat16
ALU = mybir.AluOpType
ACTF = mybir.ActivationFunctionType


def _build():
    nc = bacc.Bacc("TRN2", debug=False, num_devices=NCORES)
    z1 = nc.dram_tensor("z1", [N, D], FP32, kind="ExternalInput").ap()
    z2 = nc.dram_tensor("z2", [N, D], FP32, kind="ExternalInput").ap()
    z1T = nc.dram_tensor("z1T", [D, N], FP32, kind="ExternalInput").ap()
    z2T = nc.dram_tensor("z2T", [D, N], FP32, kind="ExternalInput").ap()
    z1l = nc.dram_tensor("z1l", [LOCAL, D], FP32, kind="ExternalInput").ap()
    z2l = nc.dram_tensor("z2l", [LOCAL, D], FP32, kind="ExternalInput").ap()
    z1lT = nc.dram_tensor("z1lT", [D, LOCAL], FP32, kind="ExternalInput").ap()
    z2lT = nc.dram_tensor("z2lT", [D, LOCAL], FP32, kind="ExternalInput").ap()
    loss = nc.dram_tensor("loss", [1, 1], FP32, kind="ExternalOutput").ap()

    with tile.TileContext(nc) as tc, ExitStack() as ctx:
        big = ctx.enter_context(tc.tile_pool(name="big", bufs=1))
        stage = ctx.enter_context(tc.tile_pool(name="stage", bufs=3))
        small = ctx.enter_context(tc.tile_pool(name="small", bufs=1))
        scratch = ctx.enter_context(tc.tile_pool(name="scratch", bufs=2))
        pmm = ctx.enter_context(tc.tile_pool(name="pmm", bufs=4, space="PSUM"))
        pbc = ctx.enter_context(tc.tile_pool(name="pbc", bufs=2, space="PSUM"))
        pcol = ctx.enter_context(tc.tile_pool(name="pcol", bufs=2, space="PSUM"))
        dram = ctx.enter_context(tc.tile_pool(name="dram", bufs=1, space="DRAM"))

        # ---- constants --------------------------------------------------
        ones_bf = small.tile([P, 1], BF16, tag="ones_bf", name="ones_bf")
        nc.vector.memset(ones_bf, 1.0)
        ones_f32 = small.tile([P, 1], FP32, tag="ones_f32", name="ones_f32")
        nc.vector.memset(ones_f32, 1.0)
        ones_row = small.tile([1, P], FP32, tag="ones_row", name="ones_row")
        nc.vector.memset(ones_row, 1.0)

        # ---- persistent operands ---------------------------------------
        ATL1 = big.tile([P, KC, LOCAL], BF16, tag="ATL1", name="ATL1")
        ATL2 = big.tile([P, KC, LOCAL], BF16, tag="ATL2", name="ATL2")
        AT1 = [
            big.tile([P, KC, 512], BF16, tag=f"AT1_{n}", name=f"AT1_{n}")
            for n in range(N_CH)
        ]
        AT2 = [
            big.tile([P, KC, 512], BF16, tag=f"AT2_{n}", name=f"AT2_{n}")
            for n in range(N_CH)
        ]
        invnb_l1 = big.tile([P, LOCAL], FP32, tag="invnb_l1", name="invnb_l1")
        invnb_l2 = big.tile([P, LOCAL], FP32, tag="invnb_l2", name="invnb_l2")

        rsp_aa = [
            small.tile([P, N_CH], FP32, tag=f"rsp_aa{m}", name=f"rsp_aa{m}")
            for m in range(M_CH)
        ]
        rsp_ab = [
            small.tile([P, N_CH], FP32, tag=f"rsp_ab{m}", name=f"rsp_ab{m}")
            for m in range(M_CH)
        ]
        rsp_bb = [
            small.tile([P, N_CH], FP32, tag=f"rsp_bb{m}", name=f"rsp_bb{m}")
            for m in range(M_CH)
        ]

        ss_l1 = small.tile([P, M_CH], FP32, tag="ss_l1", name="ss_l1")
        ss_l2 = small.tile([P, M_CH], FP32, tag="ss_l2", name="ss_l2")
        u_ab = small.tile([P, M_CH], FP32, tag="u_ab", name="u_ab")
        invn_l1 = small.tile([P, M_CH], FP32, tag="invn_l1", name="invn_l1")
        invn_l2 = small.tile([P, M_CH], FP32, tag="invn_l2", name="invn_l2")
        ss_f1 = small.tile([P, 4 * N_CH], FP32, tag="ss_f1", name="ss_f1")
        ss_f2 = small.tile([P, 4 * N_CH], FP32, tag="ss_f2", name="ss_f2")
        invn_f1 = small.tile([P, 4 * N_CH], FP32, tag="invn_f1", name="invn_f1")
        invn_f2 = small.tile([P, 4 * N_CH], FP32, tag="invn_f2", name="invn_f2")

        # DRAM scratch + collective buffers
        ivd1 = dram.tile([1, N], FP32, tag="ivd1", name="ivd1")
        ivd2 = dram.tile([1, N], FP32, tag="ivd2", name="ivd2")
        ivdl1 = dram.tile([1, LOCAL], FP32, tag="ivdl1", name="ivdl1")
        ivdl2 = dram.tile([1, LOCAL], FP32, tag="ivdl2", name="ivdl2")
        cc1_in = dram.tile([1, N], FP32, tag="cc1_in", name="cc1_in")
        cc1_out = dram.tile([M_CH, P], FP32, tag="cc1_out", name="cc1_out")
        cc2_in = dram.tile([1, 1], FP32, tag="cc2_in", name="cc2_in")
        cc2_out = dram.tile(
            [1, 1], FP32, tag="cc2_out", name="cc2_out", addr_space="Shared"
        )

        def sumsq(zt, acc_slice, nm, other=None):
            # fused (zt * 1.0) * other with row-sum on DVE
            sq = scratch.tile([P, D], FP32, tag="sq", name=f"sq_{nm}")
            nc.vector.scalar_tensor_tensor(
                out=sq, in0=zt, scalar=1.0, in1=other if other is not None else zt,
                op0=ALU.mult, op1=ALU.mult, accum_out=acc_slice,
            )

        def invn_from_ss(ss_t, invn_t, w, nm):
            # 1/sqrt(ss) = exp(-0.5*ln(ss)): stays in the exp/ln table set
            lss = scratch.tile([P, w], FP32, tag="lss", name=f"lss_{nm}")
            nc.scalar.activation(out=lss, in_=ss_t, func=ACTF.Ln)
            nc.scalar.activation(out=invn_t, in_=lss, func=ACTF.Exp, scale=-0.5)

        # ---- phase A: all row norms ------------------------------------
        # locals (also dab = a.b)
        for t in range(M_CH):
            zt1 = stage.tile([P, D], FP32, tag="st_z1", name=f"zl1_{t}")
            nc.sync.dma_start(out=zt1, in_=z1l[P * t : P * (t + 1), :])
            zt2 = stage.tile([P, D], FP32, tag="st_z2", name=f"zl2_{t}")
            nc.sync.dma_start(out=zt2, in_=z2l[P * t : P * (t + 1), :])
            sumsq(zt1, ss_l1[:, t : t + 1], f"l1_{t}")
            sumsq(zt2, ss_l2[:, t : t + 1], f"l2_{t}")
            sumsq(zt1, u_ab[:, t : t + 1], f"u_{t}", other=zt2)
        # full
        for t in range(4 * N_CH):
            zt1 = stage.tile([P, D], FP32, tag="st_z1", name=f"zf1_{t}")
            nc.sync.dma_start(out=zt1, in_=z1[P * t : P * (t + 1), :])
            sumsq(zt1, ss_f1[:, t : t + 1], f"f1_{t}")
            zt2 = stage.tile([P, D], FP32, tag="st_z2", name=f"zf2_{t}")
            nc.sync.dma_start(out=zt2, in_=z2[P * t : P * (t + 1), :])
            sumsq(zt2, ss_f2[:, t : t + 1], f"f2_{t}")

        invn_from_ss(ss_l1, invn_l1, M_CH, "l1")
        invn_from_ss(ss_l2, invn_l2, M_CH, "l2")
        invn_from_ss(ss_f1, invn_f1, 4 * N_CH, "f1")
        invn_from_ss(ss_f2, invn_f2, 4 * N_CH, "f2")

        # dab_i = u_i / (||z1_i|| * ||z2_i||)
        dab = small.tile([P, M_CH], FP32, tag="dab", name="dab")
        nc.vector.tensor_mul(dab, u_ab, invn_l1)
        nc.vector.tensor_mul(dab, dab, invn_l2)

        # spill 1/norm to DRAM in flat row order (DMA does the transpose:
        # element (p, t) of invn_* is row 128t+p)
        nc.sync.dma_start(
            out=ivd1.rearrange("o (t p) -> p (o t)", p=P), in_=invn_f1
        )
        nc.sync.dma_start(
            out=ivd2.rearrange("o (t p) -> p (o t)", p=P), in_=invn_f2
        )
        nc.sync.dma_start(
            out=ivdl1.rearrange("o (t p) -> p (o t)", p=P), in_=invn_l1
        )
        nc.sync.dma_start(
            out=ivdl2.rearrange("o (t p) -> p (o t)", p=P), in_=invn_l2
        )

        def bcast_invn(src_dram, lo, nm):
            """[1,512] slice of flat 1/norm -> PSUM [128,512] broadcast."""
            ivf = stage.tile([1, 512], FP32, tag="ivf", name=f"ivf_{nm}")
            nc.sync.dma_start(out=ivf, in_=src_dram[:, lo : lo + 512])
            pb = pbc.tile([P, 512], FP32, tag="bc", name=f"bc_{nm}")
            nc.tensor.matmul(pb, ones_row, ivf, start=True, stop=True)
            return pb

        # ---- stationary operands (local rows, d-major) ------------------
        for half in range(LOCAL // 512):
            pb1 = bcast_invn(ivdl1, 512 * half, f"l1_{half}")
            pb2 = bcast_invn(ivdl2, 512 * half, f"l2_{half}")
            nc.vector.tensor_copy(invnb_l1[:, 512 * half : 512 * (half + 1)], pb1)
            nc.vector.tensor_copy(invnb_l2[:, 512 * half : 512 * (half + 1)], pb2)
        for k in range(KC):
            zlt1 = stage.tile([P, LOCAL], FP32, tag="zlT", name=f"zlT1_{k}")
            nc.sync.dma_start(out=zlt1, in_=z1lT[P * k : P * (k + 1), :])
            nc.vector.tensor_mul(ATL1[:, k, :], zlt1, invnb_l1)
            zlt2 = stage.tile([P, LOCAL], FP32, tag="zlT", name=f"zlT2_{k}")
            nc.sync.dma_start(out=zlt2, in_=z2lT[P * k : P * (k + 1), :])
            nc.vector.tensor_mul(ATL2[:, k, :], zlt2, invnb_l2)

        # ---- phase B + main, per column chunk ---------------------------
        def prep_chunk(n):
            pb1 = bcast_invn(ivd1, 512 * n, f"f1_{n}")
            for k in range(KC):
                zt = stage.tile([P, 512], FP32, tag="st_z1", name=f"zT1_{n}_{k}")
                nc.sync.dma_start(
                    out=zt,
                    in_=z1T[P * k : P * (k + 1), 512 * n : 512 * (n + 1)],
                )
                nc.vector.tensor_mul(AT1[n][:, k, :], zt, pb1)
            pb2 = bcast_invn(ivd2, 512 * n, f"f2_{n}")
            for k in range(KC):
                zt = stage.tile([P, 512], FP32, tag="st_z2", name=f"zT2_{n}_{k}")
                nc.sync.dma_start(
                    out=zt,
                    in_=z2T[P * k : P * (k + 1), 512 * n : 512 * (n + 1)],
                )
                nc.vector.tensor_mul(AT2[n][:, k, :], zt, pb2)

        def main_chunk(n):
            colacc = scratch.tile(
                [P, 512], FP32, tag="colacc", name=f"colacc_{n}", bufs=2
            )
            for m in range(M_CH):
                aa = pmm.tile([P, 512], FP32, tag="mm", name=f"aa_{n}_{m}")
                ab = pmm.tile([P, 512], FP32, tag="mm", name=f"ab_{n}_{m}")
                bb = pmm.tile([P, 512], FP32, tag="mm", name=f"bb_{n}_{m}")
                for k in range(KC):
                    nc.tensor.matmul(
                        aa, ATL1[:, k, P * m : P * (m + 1)], AT1[n][:, k, :],
                        start=(k == 0), stop=(k == KC - 1),
                    )
                for k in range(KC):
                    nc.tensor.matmul(
                        ab, ATL1[:, k, P * m : P * (m + 1)], AT2[n][:, k, :],
                        start=(k == 0), stop=(k == KC - 1),
                    )
                for k in range(KC):
                    nc.tensor.matmul(
                        bb, ATL2[:, k, P * m : P * (m + 1)], AT2[n][:, k, :],
                        start=(k == 0), stop=(k == KC - 1),
                    )
                nc.scalar.activation(
                    out=aa, in_=aa, func=ACTF.Exp, scale=1.0 / TAU,
                    accum_out=rsp_aa[m][:, n : n + 1],
                )
                exab = scratch.tile(
                    [P, 512], BF16, tag="exab", name=f"exab_{n}_{m}", bufs=3
                )
                nc.scalar.activation(
                    out=exab, in_=ab, func=ACTF.Exp, scale=1.0 / TAU,
                    accum_out=rsp_ab[m][:, n : n + 1],
                )
                nc.scalar.activation(
                    out=bb, in_=bb, func=ACTF.Exp, scale=1.0 / TAU,
                    accum_out=rsp_bb[m][:, n : n + 1],
                )
                # column-sum accumulation on DVE (frees PE + breaks ACT->PE dep)
                if m == 0:
                    nc.vector.tensor_copy(colacc, exab)
                else:
                    nc.vector.tensor_add(colacc, colacc, exab)
            cb = scratch.tile([P, 512], BF16, tag="cb", name=f"cb_{n}")
            nc.vector.tensor_copy(cb, colacc)
            colp = pcol.tile([1, 512], FP32, tag="col", name=f"colp_{n}")
            nc.tensor.matmul(colp, ones_bf, cb, start=True, stop=True)
            csb = scratch.tile([1, 512], FP32, tag="csb", name=f"csb_{n}")
            nc.vector.tensor_copy(csb, colp)
            nc.sync.dma_start(out=cc1_in[:, 512 * n : 512 * (n + 1)], in_=csb)

        for n in range(N_CH):
            prep_chunk(n)
            main_chunk(n)

        # ---- tail -------------------------------------------------------
        rs_aa = small.tile([P, M_CH], FP32, tag="rs_aa", name="rs_aa")
        rs_ab = small.tile([P, M_CH], FP32, tag="rs_ab", name="rs_ab")
        rs_bb = small.tile([P, M_CH], FP32, tag="rs_bb", name="rs_bb")
        for m in range(M_CH):
            nc.vector.reduce_sum(
                out=rs_aa[:, m : m + 1], in_=rsp_aa[m], axis=mybir.AxisListType.X
            )
            nc.vector.reduce_sum(
                out=rs_ab[:, m : m + 1], in_=rsp_ab[m], axis=mybir.AxisListType.X
            )
            nc.vector.reduce_sum(
                out=rs_bb[:, m : m + 1], in_=rsp_bb[m], axis=mybir.AxisListType.X
            )

        denom1 = small.tile([P, M_CH], FP32, tag="denom1", name="denom1")
        nc.vector.scalar_tensor_tensor(
            out=denom1, in0=rs_aa, scalar=-EXPD, in1=rs_ab,
            op0=ALU.add, op1=ALU.add,
        )

        nc.gpsimd.collective_compute(
            "ReduceScatter",
            ALU.add,
            replica_groups=[list(range(NCORES))],
            ins=[cc1_in.opt()],
            outs=[cc1_out.opt()],
        )
        colsum_l = small.tile([P, M_CH], FP32, tag="colsum_l", name="colsum_l")
        nc.sync.dma_start(out=colsum_l, in_=cc1_out.rearrange("m p -> p m"))

        denom2 = small.tile([P, M_CH], FP32, tag="denom2", name="denom2")
        nc.vector.scalar_tensor_tensor(
            out=denom2, in0=rs_bb, scalar=-EXPD, in1=colsum_l,
            op0=ALU.add, op1=ALU.add,
        )

        nc.scalar.activation(out=denom1, in_=denom1, func=ACTF.Ln)
        nc.scalar.activation(out=denom2, in_=denom2, func=ACTF.Ln)
        nc.vector.tensor_add(denom1, denom1, denom2)  # ld1 + ld2

        combo = scratch.tile([P, M_CH], FP32, tag="combo", name="combo")
        ppart = small.tile([P, 1], FP32, tag="ppart", name="ppart")
        nc.vector.scalar_tensor_tensor(
            out=combo, in0=dab, scalar=-2.0 / TAU, in1=denom1,
            op0=ALU.mult, op1=ALU.add, accum_out=ppart,
        )
        lps = pcol.tile([1, 1], FP32, tag="col", name="lps")
        nc.tensor.matmul(lps, ones_f32, ppart, start=True, stop=True)
        lsb = small.tile([1, 1], FP32, tag="lsb", name="lsb")
        nc.scalar.mul(lsb, lps, 0.5 / N)

        nc.sync.dma_start(out=cc2_in, in_=lsb)
        nc.gpsimd.collective_compute(
            "AllReduce",
            ALU.add,
            replica_groups=[list(range(NCORES))],
            ins=[cc2_in.opt()],
            outs=[cc2_out.opt()],
        )
        nc.sync.dma_start(out=loss, in_=cc2_out)

    nc.compile()
    return nc


_NC_CACHE = None


def _get_nc():
    global _NC_CACHE
    if _NC_CACHE is None:
        _NC_CACHE = _build()
    return _NC_CACHE


def _in_maps(z1, z2):
    z1 = np.ascontiguousarray(np.asarray(z1), dtype=np.float32)
    z2 = np.ascontiguousarray(np.asarray(z2), dtype=np.float32)
    z1T = np.ascontiguousarray(z1.T)
    z2T = np.ascontiguousarray(z2.T)
    maps = []
    for c in range(NCORES):
        sl = slice(LOCAL * c, LOCAL * (c + 1))
        maps.append(
            {
                "z1": z1,
                "z2": z2,
                "z1T": z1T,
                "z2T": z2T,
                "z1l": np.ascontiguousarray(z1[sl]),
                "z2l": np.ascontiguousarray(z2[sl]),
                "z1lT": np.ascontiguousarray(z1T[:, sl]),
                "z2lT": np.ascontiguousarray(z2T[:, sl]),
            }
        )
    return maps


def kernel(z1, z2):
    nc = _get_nc()
    res = run_bass_kernel_spmd(nc, _in_maps(z1, z2), list(range(NCORES)))
    return np.asarray(res.results[0]["loss"], dtype=np.float32).reshape(())


def kernel_traced(z1, z2):
    """Same as kernel() but with NTFF profiling; returns (loss, exec_time_ns,
    trace_path)."""
    import concourse.bass_utils as bu

    bu.upload_artifacts = lambda tmpdir: "local://" + tmpdir  # no egress
    nc = _get_nc()
    res = run_bass_kernel_spmd(
        nc, _in_maps(z1, z2), list(range(NCORES)), trace=True
    )
    out = np.asarray(res.results[0]["loss"], dtype=np.float32).reshape(())
    trace_path = (
        res.instructions_and_trace[1] if res.instructions_and_trace else None
    )
    return out, res.exec_time_ns, trace_path
